# revision 1
# baseline (speedup 1.0000x reference)
"""BoT tokenizer kernel for Trainium2 (Bass/Tile), 8-core data parallel.

All 25 output tokens are computed on the TensorEngine as bf16 matmuls with
an exact fp32 -> 3x bf16 mantissa split (8+8+8 = 24 bits):

    x = a0 + a1 + a2 (each bf16, split exact by construction)
    x*w = sum_{i,j} ai*wj   (each bf16 product is exact in fp32)

 - single-feature token k: K=12 matmul (9 cross products + 3 bias rows
   against a ones column)
 - fore token: 9 features -> K = 9*9+3 = 84
 - palm token: 7 features -> K = 7*9+3 = 66

bf16 matmuls stream 1 col/cycle (vs 4 for fp32), so the PE produces each
[128,512] token tile in ~215ns. PSUM->SBUF copies are split between
VectorE and ScalarE. The kernel is then purely output-DMA bound:
each core writes 1024*25*512*4 = 52.4 MB of fp32 to HBM.
"""

import numpy as np

FORE_IDX = [0, 1, 2, 27, 28, 32, 33, 34, 38]
PALM_IDX = [4, 29, 30, 31, 35, 36, 37]
SINGLE_IDX = [3] + list(range(5, 27))

B = 8192
D = 512
T = 25
N_CORES = 8
B_LOC = B // N_CORES          # 1024 rows per core
CHUNK = 128
N_CHUNKS = B_LOC // CHUNK     # 8
ROW = T * D                   # 12800
NS = 23

# token id for single sensor k: k=0 -> token 1 (wrist), k>=1 -> token k+2
TOK_OF_SINGLE = [1] + list(range(3, 25))
# out-tile token groups for finer DMA pipelining
GROUPS = [(0, 6), (6, 12), (12, 19), (19, 25)]
KF = 9 * 9 + 3                # 84
KP = 7 * 9 + 3                # 66
KS = 12
# singles packed 3 per tile at 32-partition offsets (matmul base partition
# must be 0/32/64)
S_TILES = [(a, min(a + 3, NS)) for a in range(0, NS, 3)]
S_STRIDE = 32

_prog_cache = {}


def _k_of_tok(t):
    return 0 if t == 1 else t - 2


def _build_program():
    import concourse.bacc as bacc
    import concourse.mybir as mybir
    import concourse.tile as tile
    from concourse.bass import ts

    f32 = mybir.dt.float32
    bf16 = mybir.dt.bfloat16
    nc = bacc.Bacc("TRN2", target_bir_lowering=False, debug=False,
                   num_devices=N_CORES)

    lf_d = nc.dram_tensor("lf", [KF, B_LOC], bf16, kind="ExternalInput")
    lp_d = nc.dram_tensor("lp", [KP, B_LOC], bf16, kind="ExternalInput")
    rf_d = nc.dram_tensor("rf", [KF, D], bf16, kind="ExternalInput")
    rp_d = nc.dram_tensor("rp", [KP, D], bf16, kind="ExternalInput")
    ls_d = [nc.dram_tensor(f"ls{i}", [(b - a) * S_STRIDE, B_LOC], bf16,
                           kind="ExternalInput")
            for i, (a, b) in enumerate(S_TILES)]
    rs_d = [nc.dram_tensor(f"rs{i}", [(b - a) * S_STRIDE, D], bf16,
                           kind="ExternalInput")
            for i, (a, b) in enumerate(S_TILES)]
    out_d = nc.dram_tensor("out", [B_LOC, ROW], f32, kind="ExternalOutput")

    with tile.TileContext(nc) as tc:
        with (
            tc.tile_pool(name="cst", bufs=1) as cst,
            tc.tile_pool(name="op", bufs=3) as op,
            tc.tile_pool(name="pp", bufs=8, space="PSUM") as pp,
        ):
            lf_s = cst.tile([KF, B_LOC], bf16)
            nc.sync.dma_start(out=lf_s[:], in_=lf_d[:])
            rf_s = cst.tile([KF, D], bf16)
            nc.scalar.dma_start(out=rf_s[:], in_=rf_d[:])
            lp_s = cst.tile([KP, B_LOC], bf16)
            nc.sync.dma_start(out=lp_s[:], in_=lp_d[:])
            rp_s = cst.tile([KP, D], bf16)
            nc.scalar.dma_start(out=rp_s[:], in_=rp_d[:])
            ls_s, rs_s = [], []
            for i, (a, b) in enumerate(S_TILES):
                lt = cst.tile([(b - a) * S_STRIDE, B_LOC], bf16,
                              name=f"ls{i}_s")
                nc.sync.dma_start(out=lt[:], in_=ls_d[i][:])
                ls_s.append(lt)
                rt = cst.tile([(b - a) * S_STRIDE, D], bf16, name=f"rs{i}_s")
                nc.scalar.dma_start(out=rt[:], in_=rs_d[i][:])
                rs_s.append(rt)


            for c in range(N_CHUNKS):
                ncopy = 0
                for gi, (t0, t1) in enumerate(GROUPS):
                    o_t = op.tile([CHUNK, (t1 - t0) * D], f32, tag=f"out{gi}",
                                  bufs=4 if gi < 2 else 3)
                    for t in range(t0, t1):
                        dst = o_t[:, ts(t - t0, D)]
                        if t == 0:
                            lhsT = lf_s[:, ts(c, CHUNK)]
                            rhs = rf_s[:]
                        elif t == 2:
                            lhsT = lp_s[:, ts(c, CHUNK)]
                            rhs = rp_s[:]
                        else:
                            k = _k_of_tok(t)
                            i = k // 3
                            off = S_STRIDE * (k - S_TILES[i][0])
                            lhsT = ls_s[i][off:off + KS, ts(c, CHUNK)]
                            rhs = rs_s[i][off:off + KS, :]
                        p_t = pp.tile([CHUNK, D], f32)
                        nc.tensor.matmul(p_t[:], lhsT, rhs,
                                         start=True, stop=True)
                        if ncopy % 2 == 0:
                            nc.vector.tensor_copy(dst, p_t[:])
                        else:
                            nc.scalar.copy(dst, p_t[:])
                        ncopy += 1
                    dma_eng = nc.sync if gi % 2 == 0 else nc.scalar
                    dma_eng.dma_start(
                        out=out_d[ts(c, CHUNK), t0 * D:t1 * D], in_=o_t[:])

    nc.compile()
    return nc


def _split3(v):
    """Exact fp32 -> (bf16, bf16, bf16) mantissa split: v = s0+s1+s2."""
    import ml_dtypes
    bf = ml_dtypes.bfloat16
    v = np.asarray(v, np.float32)
    s0 = v.astype(bf)
    r1 = v - s0.astype(np.float32)
    s1 = r1.astype(bf)
    r2 = r1 - s1.astype(np.float32)
    s2 = r2.astype(bf)
    return s0, s1, s2


def _lhs_rows(xcols):
    """lhsT rows for a feature block: a0,a0,a0,a1,a1,a1,a2,a2,a2 per feat.

    xcols: [B, F] fp32 -> [9F, B] bf16"""
    import ml_dtypes
    Bn, F = xcols.shape
    s0, s1, s2 = _split3(xcols)          # each [B, F]
    out = np.empty((F, 9, Bn), dtype=ml_dtypes.bfloat16)
    for i, s in enumerate((s0, s1, s2)):
        out[:, 3 * i:3 * i + 3, :] = s.T[:, None, :]
    return out.reshape(9 * F, Bn)


def _rhs_rows(wcols):
    """rhs rows for a feature block: w0,w1,w2,w0,w1,w2,w0,w1,w2 per feat.

    wcols: [F, D] fp32 -> [9F, D] bf16"""
    import ml_dtypes
    F, Dn = wcols.shape
    s0, s1, s2 = _split3(wcols)
    out = np.empty((F, 3, 3, Dn), dtype=ml_dtypes.bfloat16)
    for j, s in enumerate((s0, s1, s2)):
        out[:, :, j, :] = s[:, None, :]
    return out.reshape(9 * F, Dn)


def _host_prep(x, Wf, bf_, Wp, bp, Ws, bs):
    import ml_dtypes
    bf16 = ml_dtypes.bfloat16

    ones3 = np.ones((3, B), dtype=bf16)

    def bias_rows(bias):
        b0, b1, b2 = _split3(bias)       # [D] each
        return np.stack([b0, b1, b2])    # [3, D]

    # fore: lhsT [84, B], rhs [84, D]
    lf = np.concatenate([_lhs_rows(x[:, FORE_IDX]), ones3])
    rf = np.concatenate([_rhs_rows(np.asarray(Wf.T)), bias_rows(bf_)])
    # palm: [66, *]
    lp = np.concatenate([_lhs_rows(x[:, PALM_IDX]), ones3])
    rp = np.concatenate([_rhs_rows(np.asarray(Wp.T)), bias_rows(bp)])

    # singles: per sensor a [12, *] block, padded to 32-partition slots
    ls_all = np.zeros((NS * S_STRIDE, B), dtype=bf16)
    rs_all = np.zeros((NS * S_STRIDE, D), dtype=bf16)
    xs = x[:, SINGLE_IDX]                # [B, 23]
    for k in range(NS):
        o = S_STRIDE * k
        ls_all[o:o + 9] = _lhs_rows(xs[:, k:k + 1])
        ls_all[o + 9:o + KS] = ones3
        rs_all[o:o + 9] = _rhs_rows(Ws[k:k + 1])
        rs_all[o + 9:o + KS] = bias_rows(bs[k])
    return lf, rf, lp, rp, ls_all, rs_all


def kernel(x, Wf, bf, Wp, bp, Ws, bs, _trace=False, _spmd_kwargs=None):
    from concourse.bass_utils import run_bass_kernel_spmd

    x = np.asarray(x, np.float32)
    lf, rf, lp, rp, ls_all, rs_all = _host_prep(
        x, np.asarray(Wf, np.float32), np.asarray(bf, np.float32),
        np.asarray(Wp, np.float32), np.asarray(bp, np.float32),
        np.asarray(Ws, np.float32), np.asarray(bs, np.float32))

    if "nc" not in _prog_cache:
        _prog_cache["nc"] = _build_program()
    nc = _prog_cache["nc"]

    in_maps = []
    for i in range(N_CORES):
        sl = slice(i * B_LOC, (i + 1) * B_LOC)
        m = {
            "lf": np.ascontiguousarray(lf[:, sl]),
            "lp": np.ascontiguousarray(lp[:, sl]),
            "rf": rf,
            "rp": rp,
        }
        for j, (a, b) in enumerate(S_TILES):
            m[f"ls{j}"] = np.ascontiguousarray(
                ls_all[S_STRIDE * a:S_STRIDE * b, sl])
            m[f"rs{j}"] = np.ascontiguousarray(rs_all[S_STRIDE * a:S_STRIDE * b])
        in_maps.append(m)

    kwargs = dict(_spmd_kwargs or {})
    res = run_bass_kernel_spmd(nc, in_maps, core_ids=list(range(N_CORES)),
                               trace=_trace, **kwargs)
    out = np.concatenate([r["out"] for r in res.results], axis=0)
    if _trace:
        kernel.last_results = res
    return out.reshape(B, T, D)



# revision 3
# speedup vs baseline: 1.2861x; 1.2861x over previous
"""BoT tokenizer kernel for Trainium2 (Bass/Tile), 8-core data parallel.

All 25 output tokens are computed on the TensorEngine as bf16 matmuls with
an exact fp32 -> 3x bf16 mantissa split (8+8+8 = 24 bits):

    x = a0 + a1 + a2 (each bf16, split exact by construction)
    x*w = sum_{i,j} ai*wj   (each bf16 product is exact in fp32)

 - single-feature token k: K=12 matmul (9 cross products + 3 bias rows
   against a ones column)
 - fore token: 9 features -> K = 9*9+3 = 84
 - palm token: 7 features -> K = 7*9+3 = 66

The output is written to HBM as bf16 (harness tolerance is 2e-2 l2; bf16
rounding is ~2.3e-3) and upcast to fp32 on the host, halving the output
DMA bytes vs fp32. PSUM->SBUF copies are done in 4-bank [128,2048] quads
(amortizing the PSUM access-latency init cost) split between VectorE and
ScalarE. Per-core HBM write: 1024*25*512*2 = 26.2 MB.
"""

import numpy as np

FORE_IDX = [0, 1, 2, 27, 28, 32, 33, 34, 38]
PALM_IDX = [4, 29, 30, 31, 35, 36, 37]
SINGLE_IDX = [3] + list(range(5, 27))

B = 8192
D = 512
T = 25
N_CORES = 8
B_LOC = B // N_CORES          # 1024 rows per core
CHUNK = 128
N_CHUNKS = B_LOC // CHUNK     # 8
ROW = T * D                   # 12800
NS = 23

# token id for single sensor k: k=0 -> token 1 (wrist), k>=1 -> token k+2
TOK_OF_SINGLE = [1] + list(range(3, 25))
# out-tile token groups (DMA granularity); quad-copies within each group
GROUPS = [(0, 8), (8, 16), (16, 25)]
# copy batches: quads of 4 tokens + the last token alone
COPY_BATCHES = [(0, 4), (4, 8), (8, 12), (12, 16), (16, 20), (20, 24),
                (24, 25)]
KF = 9 * 9 + 3                # 84
KP = 7 * 9 + 3                # 66
KS = 12
# singles packed 3 per tile at 32-partition offsets (matmul base partition
# must be 0/32/64)
S_TILES = [(a, min(a + 3, NS)) for a in range(0, NS, 3)]
S_STRIDE = 32

_prog_cache = {}


def _k_of_tok(t):
    return 0 if t == 1 else t - 2


def _build_program():
    import concourse.bacc as bacc
    import concourse.mybir as mybir
    import concourse.tile as tile
    from concourse.bass import ts

    f32 = mybir.dt.float32
    bf16 = mybir.dt.bfloat16
    nc = bacc.Bacc("TRN2", target_bir_lowering=False, debug=False,
                   num_devices=N_CORES)

    lf_d = nc.dram_tensor("lf", [KF, B_LOC], bf16, kind="ExternalInput")
    lp_d = nc.dram_tensor("lp", [KP, B_LOC], bf16, kind="ExternalInput")
    rf_d = nc.dram_tensor("rf", [KF, D], bf16, kind="ExternalInput")
    rp_d = nc.dram_tensor("rp", [KP, D], bf16, kind="ExternalInput")
    ls_d = [nc.dram_tensor(f"ls{i}", [(b - a) * S_STRIDE, B_LOC], bf16,
                           kind="ExternalInput")
            for i, (a, b) in enumerate(S_TILES)]
    rs_d = [nc.dram_tensor(f"rs{i}", [(b - a) * S_STRIDE, D], bf16,
                           kind="ExternalInput")
            for i, (a, b) in enumerate(S_TILES)]
    out_d = nc.dram_tensor("out", [B_LOC, ROW], bf16, kind="ExternalOutput")

    with tile.TileContext(nc) as tc:
        with (
            tc.tile_pool(name="cst", bufs=1) as cst,
            tc.tile_pool(name="op", bufs=3) as op,
            tc.tile_pool(name="pp", bufs=2, space="PSUM") as pp,
        ):
            lf_s = cst.tile([KF, B_LOC], bf16)
            nc.sync.dma_start(out=lf_s[:], in_=lf_d[:])
            rf_s = cst.tile([KF, D], bf16)
            nc.sync.dma_start(out=rf_s[:], in_=rf_d[:])
            lp_s = cst.tile([KP, B_LOC], bf16)
            nc.sync.dma_start(out=lp_s[:], in_=lp_d[:])
            rp_s = cst.tile([KP, D], bf16)
            nc.sync.dma_start(out=rp_s[:], in_=rp_d[:])
            ls_s, rs_s = [], []
            for i, (a, b) in enumerate(S_TILES):
                lt = cst.tile([(b - a) * S_STRIDE, B_LOC], bf16,
                              name=f"ls{i}_s")
                nc.sync.dma_start(out=lt[:], in_=ls_d[i][:])
                ls_s.append(lt)
                rt = cst.tile([(b - a) * S_STRIDE, D], bf16, name=f"rs{i}_s")
                nc.sync.dma_start(out=rt[:], in_=rs_d[i][:])
                rs_s.append(rt)

            def lhs_rhs(t, c):
                if t == 0:
                    return lf_s[:, ts(c, CHUNK)], rf_s[:]
                if t == 2:
                    return lp_s[:, ts(c, CHUNK)], rp_s[:]
                k = _k_of_tok(t)
                i = k // 3
                off = S_STRIDE * (k - S_TILES[i][0])
                return (ls_s[i][off:off + KS, ts(c, CHUNK)],
                        rs_s[i][off:off + KS, :])

            for c in range(N_CHUNKS):
                o_t = {}
                for gi, (t0, t1) in enumerate(GROUPS):
                    o_t[gi] = op.tile([CHUNK, (t1 - t0) * D], bf16,
                                      tag=f"out{gi}", name=f"out{gi}")
                nquad = 0
                for (b0, b1) in COPY_BATCHES:
                    p_t = pp.tile([CHUNK, 4 * D], f32)
                    for t in range(b0, b1):
                        lhsT, rhs = lhs_rhs(t, c)
                        nc.tensor.matmul(p_t[:, ts(t - b0, D)], lhsT, rhs,
                                         start=True, stop=True)
                    gi = next(i for i, (t0, t1) in enumerate(GROUPS)
                              if t0 <= b0 < t1)
                    g0 = GROUPS[gi][0]
                    dst = o_t[gi][:, (b0 - g0) * D:(b1 - g0) * D]
                    src = p_t[:, 0:(b1 - b0) * D]
                    if nquad % 2 == 0:
                        nc.scalar.copy(dst, src)
                    else:
                        nc.vector.tensor_copy(dst, src)
                    nquad += 1
                    if b1 in (8, 16, 25):
                        gi = {8: 0, 16: 1, 25: 2}[b1]
                        t0, t1 = GROUPS[gi]
                        nc.sync.dma_start(
                            out=out_d[ts(c, CHUNK), t0 * D:t1 * D],
                            in_=o_t[gi][:])

    nc.compile()
    return nc


def _split3(v):
    """Exact fp32 -> (bf16, bf16, bf16) mantissa split: v = s0+s1+s2."""
    import ml_dtypes
    bf = ml_dtypes.bfloat16
    v = np.asarray(v, np.float32)
    s0 = v.astype(bf)
    r1 = v - s0.astype(np.float32)
    s1 = r1.astype(bf)
    r2 = r1 - s1.astype(np.float32)
    s2 = r2.astype(bf)
    return s0, s1, s2


def _lhs_rows(xcols):
    """lhsT rows for a feature block: a0,a0,a0,a1,a1,a1,a2,a2,a2 per feat.

    xcols: [B, F] fp32 -> [9F, B] bf16"""
    import ml_dtypes
    Bn, F = xcols.shape
    s0, s1, s2 = _split3(xcols)          # each [B, F]
    out = np.empty((F, 9, Bn), dtype=ml_dtypes.bfloat16)
    for i, s in enumerate((s0, s1, s2)):
        out[:, 3 * i:3 * i + 3, :] = s.T[:, None, :]
    return out.reshape(9 * F, Bn)


def _rhs_rows(wcols):
    """rhs rows for a feature block: w0,w1,w2,w0,w1,w2,w0,w1,w2 per feat.

    wcols: [F, D] fp32 -> [9F, D] bf16"""
    import ml_dtypes
    F, Dn = wcols.shape
    s0, s1, s2 = _split3(wcols)
    out = np.empty((F, 3, 3, Dn), dtype=ml_dtypes.bfloat16)
    for j, s in enumerate((s0, s1, s2)):
        out[:, :, j, :] = s[:, None, :]
    return out.reshape(9 * F, Dn)


def _host_prep(x, Wf, bf_, Wp, bp, Ws, bs):
    import ml_dtypes
    bf16 = ml_dtypes.bfloat16

    ones3 = np.ones((3, B), dtype=bf16)

    def bias_rows(bias):
        b0, b1, b2 = _split3(bias)       # [D] each
        return np.stack([b0, b1, b2])    # [3, D]

    # fore: lhsT [84, B], rhs [84, D]
    lf = np.concatenate([_lhs_rows(x[:, FORE_IDX]), ones3])
    rf = np.concatenate([_rhs_rows(np.asarray(Wf.T)), bias_rows(bf_)])
    # palm: [66, *]
    lp = np.concatenate([_lhs_rows(x[:, PALM_IDX]), ones3])
    rp = np.concatenate([_rhs_rows(np.asarray(Wp.T)), bias_rows(bp)])

    # singles: per sensor a [12, *] block, padded to 32-partition slots
    ls_all = np.zeros((NS * S_STRIDE, B), dtype=bf16)
    rs_all = np.zeros((NS * S_STRIDE, D), dtype=bf16)
    xs = x[:, SINGLE_IDX]                # [B, 23]
    for k in range(NS):
        o = S_STRIDE * k
        ls_all[o:o + 9] = _lhs_rows(xs[:, k:k + 1])
        ls_all[o + 9:o + KS] = ones3
        rs_all[o:o + 9] = _rhs_rows(Ws[k:k + 1])
        rs_all[o + 9:o + KS] = bias_rows(bs[k])
    return lf, rf, lp, rp, ls_all, rs_all


def kernel(x, Wf, bf, Wp, bp, Ws, bs, _trace=False, _spmd_kwargs=None):
    from concourse.bass_utils import run_bass_kernel_spmd

    x = np.asarray(x, np.float32)
    lf, rf, lp, rp, ls_all, rs_all = _host_prep(
        x, np.asarray(Wf, np.float32), np.asarray(bf, np.float32),
        np.asarray(Wp, np.float32), np.asarray(bp, np.float32),
        np.asarray(Ws, np.float32), np.asarray(bs, np.float32))

    if "nc" not in _prog_cache:
        _prog_cache["nc"] = _build_program()
    nc = _prog_cache["nc"]

    in_maps = []
    for i in range(N_CORES):
        sl = slice(i * B_LOC, (i + 1) * B_LOC)
        m = {
            "lf": np.ascontiguousarray(lf[:, sl]),
            "lp": np.ascontiguousarray(lp[:, sl]),
            "rf": rf,
            "rp": rp,
        }
        for j, (a, b) in enumerate(S_TILES):
            m[f"ls{j}"] = np.ascontiguousarray(
                ls_all[S_STRIDE * a:S_STRIDE * b, sl])
            m[f"rs{j}"] = np.ascontiguousarray(rs_all[S_STRIDE * a:S_STRIDE * b])
        in_maps.append(m)

    kwargs = dict(_spmd_kwargs or {})
    res = run_bass_kernel_spmd(nc, in_maps, core_ids=list(range(N_CORES)),
                               trace=_trace, **kwargs)
    out = np.concatenate([np.asarray(r["out"]) for r in res.results], axis=0)
    if _trace:
        kernel.last_results = res
    return out.astype(np.float32).reshape(B, T, D)


# revision 6
# speedup vs baseline: 1.5864x; 1.2335x over previous
"""BoT tokenizer kernel for Trainium2 (Bass/Tile), 8-core data parallel.

Output tokens are produced by two parallel paths:

 - PE path (tokens 0-14): bf16 matmuls with an exact fp32 -> 3x bf16
   mantissa split (8+8+8 = 24 bits), PSUM -> SBUF pair-copies ([128,1024],
   amortizing PSUM access latency) split between ScalarE and VectorE.
 - Direct path (tokens 15-24, all rank-1 single-sensor tokens):
   out[p,f] = xs[p] * W[f] computed as one DVE tensor_scalar_mul per token
   (all-SBUF bf16 -> 4x DVE rate), then the bias added with two big
   [128, 5*512] tensor_tensor adds: one on DVE (2x) and one on GpSimd.
   No PSUM traffic, and it keeps the TensorEngine load small enough that
   its low DVFS p-state does not become the critical path.

The output is written to HBM as bf16 (harness tolerance is 2e-2 l2; bf16
rounding is ~2.3e-3) and upcast to fp32 on the host, halving output DMA
bytes vs fp32. Per-core HBM write: 1024*25*512*2 = 26.2 MB.
"""

import numpy as np

FORE_IDX = [0, 1, 2, 27, 28, 32, 33, 34, 38]
PALM_IDX = [4, 29, 30, 31, 35, 36, 37]
SINGLE_IDX = [3] + list(range(5, 27))

B = 8192
D = 512
T = 25
N_CORES = 8
B_LOC = B // N_CORES          # 1024 rows per core
CHUNK = 128
N_CHUNKS = B_LOC // CHUNK     # 8
ROW = T * D                   # 12800
NS = 23

# token id for single sensor k: k=0 -> token 1 (wrist), k>=1 -> token k+2
TOK_OF_SINGLE = [1] + list(range(3, 25))
# tokens >= DIR_T0 go through the DVE/GpSimd direct path
DIR_T0 = 15
N_DIR = T - DIR_T0            # 10
# out-tile token groups (DMA granularity)
GROUPS = [(0, 8), (8, DIR_T0), (DIR_T0, 25)]
# PE-path copy batches: pairs + trailing single; engine: 'a'=Act, 'v'=DVE
COPY_BATCHES = [(0, 2, 'a'), (2, 4, 'v'), (4, 6, 'a'), (6, 8, 'a'),
                (8, 10, 'v'), (10, 12, 'a'), (12, 14, 'a'), (14, 15, 'a')]
KF = 9 * 9 + 3                # 84
KP = 7 * 9 + 3                # 66
KS = 12
# PE-path singles packed 3 per tile at 32-partition offsets (matmul base
# partition must be 0/32/64); sensors 0..12 (tokens 1, 3..14) use it
S_TILES = [(a, min(a + 3, NS)) for a in range(0, NS, 3)]
S_STRIDE = 32
N_PE_TILES = 5

_prog_cache = {}


def _k_of_tok(t):
    return 0 if t == 1 else t - 2


def _build_program():
    import concourse.bacc as bacc
    import concourse.mybir as mybir
    import concourse.tile as tile
    from concourse.bass import ts

    f32 = mybir.dt.float32
    bf16 = mybir.dt.bfloat16
    add = mybir.AluOpType.add
    nc = bacc.Bacc("TRN2", target_bir_lowering=False, debug=False,
                   num_devices=N_CORES)

    lf_d = nc.dram_tensor("lf", [KF, B_LOC], bf16, kind="ExternalInput")
    lp_d = nc.dram_tensor("lp", [KP, B_LOC], bf16, kind="ExternalInput")
    rf_d = nc.dram_tensor("rf", [KF, D], bf16, kind="ExternalInput")
    rp_d = nc.dram_tensor("rp", [KP, D], bf16, kind="ExternalInput")
    ls_d = [nc.dram_tensor(f"ls{i}", [(b - a) * S_STRIDE, B_LOC], bf16,
                           kind="ExternalInput")
            for i, (a, b) in enumerate(S_TILES[:N_PE_TILES])]
    rs_d = [nc.dram_tensor(f"rs{i}", [(b - a) * S_STRIDE, D], bf16,
                           kind="ExternalInput")
            for i, (a, b) in enumerate(S_TILES[:N_PE_TILES])]
    # direct-path: [W_15..W_24 | b_15..b_24] broadcast rows + scalar columns
    wb_d = nc.dram_tensor("wb", [CHUNK, N_DIR * 2 * D], bf16,
                          kind="ExternalInput")
    xs_d = nc.dram_tensor("xs", [CHUNK, N_CHUNKS * N_DIR], f32,
                          kind="ExternalInput")
    out_d = nc.dram_tensor("out", [B_LOC, ROW], bf16, kind="ExternalOutput")

    with tile.TileContext(nc) as tc:
        with (
            tc.tile_pool(name="cst", bufs=1) as cst,
            tc.tile_pool(name="op", bufs=3) as op,
            tc.tile_pool(name="pp", bufs=4, space="PSUM") as pp,
        ):
            wb_s = cst.tile([CHUNK, N_DIR * 2 * D], bf16)
            nc.sync.dma_start(out=wb_s[:], in_=wb_d[:])
            xs_s = cst.tile([CHUNK, N_CHUNKS * N_DIR], f32)
            nc.sync.dma_start(out=xs_s[:], in_=xs_d[:])
            lf_s = cst.tile([KF, B_LOC], bf16)
            nc.sync.dma_start(out=lf_s[:], in_=lf_d[:])
            rf_s = cst.tile([KF, D], bf16)
            nc.sync.dma_start(out=rf_s[:], in_=rf_d[:])
            lp_s = cst.tile([KP, B_LOC], bf16)
            nc.sync.dma_start(out=lp_s[:], in_=lp_d[:])
            rp_s = cst.tile([KP, D], bf16)
            nc.sync.dma_start(out=rp_s[:], in_=rp_d[:])
            ls_s, rs_s = [], []
            for i in range(N_PE_TILES):
                a, b = S_TILES[i]
                lt = cst.tile([(b - a) * S_STRIDE, B_LOC], bf16,
                              name=f"ls{i}_s")
                nc.sync.dma_start(out=lt[:], in_=ls_d[i][:])
                ls_s.append(lt)
                rt = cst.tile([(b - a) * S_STRIDE, D], bf16, name=f"rs{i}_s")
                nc.sync.dma_start(out=rt[:], in_=rs_d[i][:])
                rs_s.append(rt)

            def lhs_rhs(t, c):
                if t == 0:
                    return lf_s[:, ts(c, CHUNK)], rf_s[:]
                if t == 2:
                    return lp_s[:, ts(c, CHUNK)], rp_s[:]
                k = _k_of_tok(t)
                i = k // 3
                off = S_STRIDE * (k - S_TILES[i][0])
                return (ls_s[i][off:off + KS, ts(c, CHUNK)],
                        rs_s[i][off:off + KS, :])

            half = (N_DIR // 2) * D      # 2560
            for c in range(N_CHUNKS):
                o_t = {}
                for gi, (t0, t1) in enumerate(GROUPS):
                    o_t[gi] = op.tile([CHUNK, (t1 - t0) * D], bf16,
                                      tag=f"out{gi}", name=f"out{gi}")
                # direct path: tokens DIR_T0..24, no PSUM
                for j in range(N_DIR):
                    col = c * N_DIR + j
                    nc.vector.tensor_scalar_mul(
                        o_t[2][:, ts(j, D)], wb_s[:, ts(j, D)],
                        xs_s[:, col:col + 1])
                bb = wb_s[:, N_DIR * D:2 * N_DIR * D]
                nc.vector.tensor_tensor(
                    o_t[2][:, 0:half], o_t[2][:, 0:half], bb[:, 0:half],
                    op=add)
                nc.gpsimd.tensor_tensor(
                    o_t[2][:, half:N_DIR * D], o_t[2][:, half:N_DIR * D],
                    bb[:, half:N_DIR * D], op=add)
                nc.sync.dma_start(
                    out=out_d[ts(c, CHUNK), GROUPS[2][0] * D:GROUPS[2][1] * D],
                    in_=o_t[2][:])
                # PE path: tokens 0..14, pair-copied PSUM -> SBUF
                for (b0, b1, eng) in COPY_BATCHES:
                    p_t = pp.tile([CHUNK, 2 * D], f32)
                    for t in range(b0, b1):
                        lhsT, rhs = lhs_rhs(t, c)
                        nc.tensor.matmul(p_t[:, ts(t - b0, D)], lhsT, rhs,
                                         start=True, stop=True)
                    gi = 0 if b0 < 8 else 1
                    g0 = GROUPS[gi][0]
                    dst = o_t[gi][:, (b0 - g0) * D:(b1 - g0) * D]
                    src = p_t[:, 0:(b1 - b0) * D]
                    if eng == 'a':
                        nc.scalar.copy(dst, src)
                    else:
                        nc.vector.tensor_copy(dst, src)
                    if b1 in (8, DIR_T0):
                        gi = 0 if b1 == 8 else 1
                        t0, t1 = GROUPS[gi]
                        nc.sync.dma_start(
                            out=out_d[ts(c, CHUNK), t0 * D:t1 * D],
                            in_=o_t[gi][:])

    nc.compile()
    return nc


def _split3(v):
    """Exact fp32 -> (bf16, bf16, bf16) mantissa split: v = s0+s1+s2."""
    import ml_dtypes
    bf = ml_dtypes.bfloat16
    v = np.asarray(v, np.float32)
    s0 = v.astype(bf)
    r1 = v - s0.astype(np.float32)
    s1 = r1.astype(bf)
    r2 = r1 - s1.astype(np.float32)
    s2 = r2.astype(bf)
    return s0, s1, s2


def _lhs_rows(xcols):
    """lhsT rows for a feature block: a0,a0,a0,a1,a1,a1,a2,a2,a2 per feat.

    xcols: [B, F] fp32 -> [9F, B] bf16"""
    import ml_dtypes
    Bn, F = xcols.shape
    s0, s1, s2 = _split3(xcols)          # each [B, F]
    out = np.empty((F, 9, Bn), dtype=ml_dtypes.bfloat16)
    for i, s in enumerate((s0, s1, s2)):
        out[:, 3 * i:3 * i + 3, :] = s.T[:, None, :]
    return out.reshape(9 * F, Bn)


def _rhs_rows(wcols):
    """rhs rows for a feature block: w0,w1,w2,w0,w1,w2,w0,w1,w2 per feat.

    wcols: [F, D] fp32 -> [9F, D] bf16"""
    import ml_dtypes
    F, Dn = wcols.shape
    s0, s1, s2 = _split3(wcols)
    out = np.empty((F, 3, 3, Dn), dtype=ml_dtypes.bfloat16)
    for j, s in enumerate((s0, s1, s2)):
        out[:, :, j, :] = s[:, None, :]
    return out.reshape(9 * F, Dn)


def _host_prep(x, Wf, bf_, Wp, bp, Ws, bs):
    import ml_dtypes
    bf16 = ml_dtypes.bfloat16

    ones3 = np.ones((3, B), dtype=bf16)

    def bias_rows(bias):
        b0, b1, b2 = _split3(bias)       # [D] each
        return np.stack([b0, b1, b2])    # [3, D]

    # fore: lhsT [84, B], rhs [84, D]
    lf = np.concatenate([_lhs_rows(x[:, FORE_IDX]), ones3])
    rf = np.concatenate([_rhs_rows(np.asarray(Wf.T)), bias_rows(bf_)])
    # palm: [66, *]
    lp = np.concatenate([_lhs_rows(x[:, PALM_IDX]), ones3])
    rp = np.concatenate([_rhs_rows(np.asarray(Wp.T)), bias_rows(bp)])

    # PE-path singles: tokens 1, 3..DIR_T0-1 -> sensors 0..DIR_T0-3
    n_pe_sing = 1 + (DIR_T0 - 3)
    ls_all = np.zeros((NS * S_STRIDE, B), dtype=bf16)
    rs_all = np.zeros((NS * S_STRIDE, D), dtype=bf16)
    xs = x[:, SINGLE_IDX]                # [B, 23]
    for k in range(n_pe_sing):
        o = S_STRIDE * k
        ls_all[o:o + 9] = _lhs_rows(xs[:, k:k + 1])
        ls_all[o + 9:o + KS] = ones3
        rs_all[o:o + 9] = _rhs_rows(Ws[k:k + 1])
        rs_all[o + 9:o + KS] = bias_rows(bs[k])

    # direct-path broadcast rows: [W_15..24 | b_15..24]
    wb = np.empty((CHUNK, N_DIR * 2 * D), dtype=bf16)
    for j in range(N_DIR):
        k = _k_of_tok(DIR_T0 + j)
        wb[:, j * D:(j + 1) * D] = np.asarray(
            Ws[k], np.float32).astype(bf16)[None, :]
        wb[:, (N_DIR + j) * D:(N_DIR + j + 1) * D] = np.asarray(
            bs[k], np.float32).astype(bf16)[None, :]
    return lf, rf, lp, rp, ls_all, rs_all, wb, xs


def kernel(x, Wf, bf, Wp, bp, Ws, bs, _trace=False, _spmd_kwargs=None):
    from concourse.bass_utils import run_bass_kernel_spmd

    x = np.asarray(x, np.float32)
    lf, rf, lp, rp, ls_all, rs_all, wb, xs = _host_prep(
        x, np.asarray(Wf, np.float32), np.asarray(bf, np.float32),
        np.asarray(Wp, np.float32), np.asarray(bp, np.float32),
        np.asarray(Ws, np.float32), np.asarray(bs, np.float32))

    if "nc" not in _prog_cache:
        _prog_cache["nc"] = _build_program()
    nc = _prog_cache["nc"]

    in_maps = []
    for i in range(N_CORES):
        sl = slice(i * B_LOC, (i + 1) * B_LOC)
        # per-chunk scalar columns for the direct path
        xs_loc = xs[sl]                  # [B_LOC, 23] fp32
        xs_cols = np.empty((CHUNK, N_CHUNKS * N_DIR), np.float32)
        for c in range(N_CHUNKS):
            for j in range(N_DIR):
                k = _k_of_tok(DIR_T0 + j)
                xs_cols[:, c * N_DIR + j] = xs_loc[c * CHUNK:(c + 1) * CHUNK, k]
        m = {
            "lf": np.ascontiguousarray(lf[:, sl]),
            "lp": np.ascontiguousarray(lp[:, sl]),
            "rf": rf,
            "rp": rp,
            "wb": wb,
            "xs": xs_cols,
        }
        for j in range(N_PE_TILES):
            a, b = S_TILES[j]
            m[f"ls{j}"] = np.ascontiguousarray(
                ls_all[S_STRIDE * a:S_STRIDE * b, sl])
            m[f"rs{j}"] = np.ascontiguousarray(rs_all[S_STRIDE * a:S_STRIDE * b])
        in_maps.append(m)

    kwargs = dict(_spmd_kwargs or {})
    res = run_bass_kernel_spmd(nc, in_maps, core_ids=list(range(N_CORES)),
                               trace=_trace, **kwargs)
    out = np.concatenate([np.asarray(r["out"]) for r in res.results], axis=0)
    if _trace:
        kernel.last_results = res
    return out.astype(np.float32).reshape(B, T, D)


# revision 8
# speedup vs baseline: 1.9734x; 1.2439x over previous
"""BoT tokenizer kernel for Trainium2 (Bass/Tile), 8-core data parallel.

All 25 output tokens are computed on the TensorEngine as fp8 (e4m3)
DoubleRow matmuls: the moving stream runs at 2 rows/cycle, halving PE
time vs bf16 so the PE keeps up even at its low DVFS p-state.

fp32 operands are split into 3 fp8 terms (x = a0+a1+a2, ~4 bits each) and
the matmul accumulates the 6 dominant cross products a_i*w_j (i+j <= 2),
each product pair scaled by (2^(2i-2j), 2^(2j-2i)) to keep every fp8 row
in the normal range; 2 extra rows add the bias (b0 at 2^-6, residual at
2^-8). Achieved accuracy: ~1.8e-3 l2 before output rounding.

 - single-sensor token: K = 6+2 = 8 logical rows -> Kp=4 partitions
 - fore token: K = 9*6+2 = 56 -> Kp=28;  palm: 7*6+2 = 44 -> Kp=22

DoubleRow layout: logical row k lives at partition k//2, half k%2, i.e.
lhsT tiles are [Kp, 2, B], rhs tiles [Kp, 2, D].

PSUM -> SBUF pair-copies ([128,1024], amortizing PSUM access latency)
split between ScalarE and VectorE. The output is written to HBM as bf16
(harness tolerance 2e-2 l2; this kernel ~2.4e-3) and upcast to fp32 on
the host, halving output DMA bytes. Per-core HBM write: 26.2 MB.
"""

import numpy as np

FORE_IDX = [0, 1, 2, 27, 28, 32, 33, 34, 38]
PALM_IDX = [4, 29, 30, 31, 35, 36, 37]
SINGLE_IDX = [3] + list(range(5, 27))

B = 8192
D = 512
T = 25
N_CORES = 8
B_LOC = B // N_CORES          # 1024 rows per core
CHUNK = 128
N_CHUNKS = B_LOC // CHUNK     # 8
ROW = T * D                   # 12800
NS = 23

# token id for single sensor k: k=0 -> token 1 (wrist), k>=1 -> token k+2
TOK_OF_SINGLE = [1] + list(range(3, 25))
# out-tile token groups (DMA granularity)
GROUPS = [(0, 8), (8, 16), (16, 25)]
# copy batches: (t0, t1, engine) pairs + trailing single
COPY_BATCHES = [(0, 2, 'a'), (2, 4, 'v'), (4, 6, 'a'), (6, 8, 'v'),
                (8, 10, 'a'), (10, 12, 'v'), (12, 14, 'a'), (14, 16, 'v'),
                (16, 18, 'a'), (18, 20, 'v'), (20, 22, 'a'), (22, 24, 'v'),
                (24, 25, 'a')]
# cross products (i,j) of the 3-term fp8 splits kept in the contraction
PRODS = [(0, 0), (0, 1), (1, 0), (1, 1), (0, 2), (2, 0)]
KPF = (9 * 6 + 2) // 2        # 28 partitions, fore
KPP = (7 * 6 + 2) // 2        # 22 partitions, palm
KPS = (6 + 2) // 2            # 4 partitions, single
# singles packed 3 per tile at 32-partition offsets (matmul base partition
# must be 0/32/64)
S_TILES = [(a, min(a + 3, NS)) for a in range(0, NS, 3)]   # 8 tiles
S_STRIDE = 32

_prog_cache = {}


def _k_of_tok(t):
    return 0 if t == 1 else t - 2


def _build_program():
    import concourse.bacc as bacc
    import concourse.mybir as mybir
    import concourse.tile as tile
    from concourse.bass import ts

    f32 = mybir.dt.float32
    bf16 = mybir.dt.bfloat16
    fp8 = mybir.dt.float8e4
    dr = mybir.MatmulPerfMode.DoubleRow
    nc = bacc.Bacc("TRN2", target_bir_lowering=False, debug=False,
                   num_devices=N_CORES)

    lf_d = nc.dram_tensor("lf", [KPF, 2, B_LOC], fp8, kind="ExternalInput")
    lp_d = nc.dram_tensor("lp", [KPP, 2, B_LOC], fp8, kind="ExternalInput")
    rf_d = nc.dram_tensor("rf", [KPF, 2, D], fp8, kind="ExternalInput")
    rp_d = nc.dram_tensor("rp", [KPP, 2, D], fp8, kind="ExternalInput")
    ls_d = [nc.dram_tensor(f"ls{i}", [96, 2, B_LOC], fp8,
                           kind="ExternalInput")
            for i in range(len(S_TILES))]
    rs_d = [nc.dram_tensor(f"rs{i}", [96, 2, D], fp8,
                           kind="ExternalInput")
            for i in range(len(S_TILES))]
    out_d = nc.dram_tensor("out", [B_LOC, ROW], bf16, kind="ExternalOutput")

    with tile.TileContext(nc) as tc:
        with (
            tc.tile_pool(name="cst", bufs=1) as cst,
            tc.tile_pool(name="op", bufs=3) as op,
            tc.tile_pool(name="pp", bufs=4, space="PSUM") as pp,
        ):
            lf_s = cst.tile([KPF, 2, B_LOC], fp8)
            nc.sync.dma_start(out=lf_s[:], in_=lf_d[:])
            rf_s = cst.tile([KPF, 2, D], fp8)
            nc.sync.dma_start(out=rf_s[:], in_=rf_d[:])
            lp_s = cst.tile([KPP, 2, B_LOC], fp8)
            nc.sync.dma_start(out=lp_s[:], in_=lp_d[:])
            rp_s = cst.tile([KPP, 2, D], fp8)
            nc.sync.dma_start(out=rp_s[:], in_=rp_d[:])
            ls_s, rs_s = [], []
            for i in range(len(S_TILES)):
                lt = cst.tile([96, 2, B_LOC], fp8, name=f"ls{i}_s")
                nc.sync.dma_start(out=lt[:], in_=ls_d[i][:])
                ls_s.append(lt)
                rt = cst.tile([96, 2, D], fp8, name=f"rs{i}_s")
                nc.sync.dma_start(out=rt[:], in_=rs_d[i][:])
                rs_s.append(rt)

            def lhs_rhs(t, c):
                if t == 0:
                    return lf_s[:, :, ts(c, CHUNK)], rf_s[:]
                if t == 2:
                    return lp_s[:, :, ts(c, CHUNK)], rp_s[:]
                k = _k_of_tok(t)
                i = k // 3
                off = S_STRIDE * (k % 3)
                return (ls_s[i][off:off + KPS, :, ts(c, CHUNK)],
                        rs_s[i][off:off + KPS, :, :])

            for c in range(N_CHUNKS):
                o_t = {}
                for gi, (t0, t1) in enumerate(GROUPS):
                    o_t[gi] = op.tile([CHUNK, (t1 - t0) * D], bf16,
                                      tag=f"out{gi}", name=f"out{gi}")
                for (b0, b1, eng) in COPY_BATCHES:
                    p_t = pp.tile([CHUNK, 2 * D], f32)
                    for t in range(b0, b1):
                        lhsT, rhs = lhs_rhs(t, c)
                        nc.tensor.matmul(p_t[:, ts(t - b0, D)], lhsT, rhs,
                                         start=True, stop=True, perf_mode=dr)
                    gi = next(i for i, (t0, t1) in enumerate(GROUPS)
                              if t0 <= b0 < t1)
                    g0 = GROUPS[gi][0]
                    dst = o_t[gi][:, (b0 - g0) * D:(b1 - g0) * D]
                    src = p_t[:, 0:(b1 - b0) * D]
                    if eng == 'a':
                        nc.scalar.copy(dst, src)
                    else:
                        nc.vector.tensor_copy(dst, src)
                    if b1 in (8, 16, 25):
                        gi = {8: 0, 16: 1, 25: 2}[b1]
                        t0, t1 = GROUPS[gi]
                        nc.sync.dma_start(
                            out=out_d[ts(c, CHUNK), t0 * D:t1 * D],
                            in_=o_t[gi][:])

    nc.compile()
    return nc


def _split3_f8(v):
    """fp32 -> 3 fp8(e4m3) terms: v ~= a0+a1+a2."""
    import ml_dtypes
    f8 = ml_dtypes.float8_e4m3
    v = np.asarray(v, np.float32)
    a0 = v.astype(f8)
    r1 = v - a0.astype(np.float32)
    a1 = r1.astype(f8)
    r2 = r1 - a1.astype(np.float32)
    a2 = r2.astype(f8)
    return a0, a1, a2


def _build_pair(xcols, wrows, bias):
    """lhs/rhs row stacks for one token's DoubleRow matmul.

    xcols: [B, F] fp32 (features), wrows: [F, D] fp32, bias: [D] fp32.
    Returns lhs [Kp, 2, B] fp8, rhs [Kp, 2, D] fp8 with K = 6F+2."""
    import ml_dtypes
    f8 = ml_dtypes.float8_e4m3
    Bn, F = xcols.shape
    K = 6 * F + 2
    lhs = np.zeros((K, Bn), dtype=f8)
    rhs = np.zeros((K, D), dtype=f8)
    ax = _split3_f8(xcols)               # each [B, F]
    aw = _split3_f8(wrows)               # each [F, D]
    for f in range(F):
        for p, (i, j) in enumerate(PRODS):
            k = f * 6 + p
            lam = np.float32(2.0 ** (2 * i - 2 * j))
            lhs[k] = (ax[i][:, f].astype(np.float32) * lam).astype(f8)
            rhs[k] = (aw[j][f].astype(np.float32) / lam).astype(f8)
    b0 = (bias * 2.0 ** 6).astype(f8)
    rb = bias - b0.astype(np.float32) * 2.0 ** -6
    b1 = (rb * 2.0 ** 8).astype(f8)
    lhs[6 * F] = np.float32(2.0 ** -6)
    rhs[6 * F] = b0
    lhs[6 * F + 1] = np.float32(2.0 ** -8)
    rhs[6 * F + 1] = b1
    # DoubleRow packing: logical row k -> (partition k//2, half k%2)
    return (lhs.reshape(K // 2, 2, Bn), rhs.reshape(K // 2, 2, D))


def _host_prep(x, Wf, bf_, Wp, bp, Ws, bs):
    import ml_dtypes
    f8 = ml_dtypes.float8_e4m3

    lf, rf = _build_pair(x[:, FORE_IDX], np.asarray(Wf.T), bf_)
    lp, rp = _build_pair(x[:, PALM_IDX], np.asarray(Wp.T), bp)

    xs = x[:, SINGLE_IDX]                # [B, 23]
    ls_all = np.zeros((len(S_TILES), 96, 2, B), dtype=f8)
    rs_all = np.zeros((len(S_TILES), 96, 2, D), dtype=f8)
    for k in range(NS):
        i, off = k // 3, S_STRIDE * (k % 3)
        lsk, rsk = _build_pair(xs[:, k:k + 1], Ws[k:k + 1], bs[k])
        ls_all[i, off:off + KPS] = lsk
        rs_all[i, off:off + KPS] = rsk
    return lf, rf, lp, rp, ls_all, rs_all


def kernel(x, Wf, bf, Wp, bp, Ws, bs, _trace=False, _spmd_kwargs=None):
    from concourse.bass_utils import run_bass_kernel_spmd

    x = np.asarray(x, np.float32)
    lf, rf, lp, rp, ls_all, rs_all = _host_prep(
        x, np.asarray(Wf, np.float32), np.asarray(bf, np.float32),
        np.asarray(Wp, np.float32), np.asarray(bp, np.float32),
        np.asarray(Ws, np.float32), np.asarray(bs, np.float32))

    if "nc" not in _prog_cache:
        _prog_cache["nc"] = _build_program()
    nc = _prog_cache["nc"]

    in_maps = []
    for i in range(N_CORES):
        sl = slice(i * B_LOC, (i + 1) * B_LOC)
        m = {
            "lf": np.ascontiguousarray(lf[:, :, sl]),
            "lp": np.ascontiguousarray(lp[:, :, sl]),
            "rf": rf,
            "rp": rp,
        }
        for j in range(len(S_TILES)):
            m[f"ls{j}"] = np.ascontiguousarray(ls_all[j][:, :, sl])
            m[f"rs{j}"] = rs_all[j]
        in_maps.append(m)

    kwargs = dict(_spmd_kwargs or {})
    res = run_bass_kernel_spmd(nc, in_maps, core_ids=list(range(N_CORES)),
                               trace=_trace, **kwargs)
    out = np.concatenate([np.asarray(r["out"]) for r in res.results], axis=0)
    if _trace:
        kernel.last_results = res
    return out.astype(np.float32).reshape(B, T, D)
